# revision 19
# baseline (speedup 1.0000x reference)
"""Trainium2 Bass kernel for nn_EntropyCalculator (per-row histogram entropy).

x: [262144, 64] ints in [0, 40). Output: [262144, 1] float32 per-row entropy
of the value histogram: -sum_v p_v*log(p_v + 1e-8), p = c/(64+1e-8).

Strategy (per core, pure data parallel over 8 cores):
  The 40-bin histogram is computed with 20 "pair limb" passes on the DVE.
  Pass g consumes TWO elements per cycle (even/odd streams on the two SBUF
  read ports) and accumulates d(a) + d(b) into a running prefix, where
  d(x) = 1 if x == 2g, 128 if x == 2g+1, else 0 (an 8-stage hand-placed
  uop program: IS_EQ, IS_EQ, SELECT per element + pair-add + scan).
  Per-row sums are recovered by strided differences of the prefix at row
  boundaries; digits c0 = count(2g), c1 = count(2g+1) are split with an
  exact magic-number rint on the ACT engine, and the entropy tail
  (Ln + c*u dot) runs on ACT/GPSIMD/DVE.

  Exactness: pair values are integers <= 256; chunk prefixes < 2^24, so
  every fp32 partial sum is exact.  A = c0 + 128*c1 with c0, c1 <= 64;
  c1 = rint(A/128 - 0.25) exactly (c0/128 - 0.25 in [-0.25, 0.25]).
"""

import numpy as np

VOCAB = 40
L = 64
B = 262144
NCORES = 8
ROWS_PC = B // NCORES          # 32768 rows per core
P = 128                        # SBUF partitions
RPP = ROWS_PC // P             # 256 rows per partition
RC = 64                        # rows per partition per chunk
NCHUNK = RPP // RC             # 4 chunks
PAIRS = L // 2                 # 32 element-pairs per row
NL = VOCAB // 2                # 20 pair limbs
W = 128.0                      # digit weight for the odd bin of a limb
EPS = 1e-8
S_PRIME = 64.0 + EPS
MAGIC = 8388608.0              # 2^23
SEGMENTED = False              # scan resets per row (measured slower: uop
                               # transitions cost ~3 cy/row) vs boundary diffs

_RUNNER = None


def _build_pair2_uops():
    """Hand-placed 8-stage uop program for the PAIR2 op:

        out[k] = scan_add( d(in0[k]) + d(in1[k]) ),
        d(x) = select(x == s1, imm2, x == s0)

    Two elements per cycle (both read ports), fused map+pair-add+scan.
    The stock scheduler cannot place this in 8 stages (select-cond shims);
    hand placement puts each SELECT's cond at stage-1 via PREV_ALU_OUT.
    """
    from concourse.dve_uop import (
        UopConfig, UopDpConfig, AluInp, AluOp, DelayInp, InpSel, OutPath,
        OutSel, Trigger, ENABLE, DISABLE,
    )

    D0, D1, D2, D3, D4, D5 = (
        AluInp.PREV_DELAY_0, AluInp.PREV_DELAY_1, AluInp.PREV_DELAY_2,
        AluInp.PREV_DELAY_3, AluInp.PREV_DELAY_4, AluInp.PREV_DELAY_5,
    )
    PREV = AluInp.PREV_ALU_OUT
    CURR = AluInp.CURR_ALU_OUT

    def dp_stages(kind: str):
        # kind: "seed" (acc <- 0), "steady" (acc += pair),
        #       "step" (acc <- 0 + pair; fires on row boundary)
        dp = [UopDpConfig() for _ in range(8)]
        for st in range(8):
            dp[st].pass_through_delay(0, 1, 2, 3, 4, 5)
        # st0: e0  = IS_EQ(src0, c0)
        dp[0].enable_alu(AluOp.IS_EQ, D0, D2)
        # st1: c1  = IS_EQ(src0, c1const); capture e0 -> lane0
        dp[1].enable_alu(AluOp.IS_EQ, D0, D3)
        dp[1].enable_delay_from_src(DelayInp.PREV_ALU_OUT, 0)
        # st2: selE = SELECT(cond=c1 via PREV, else=e0, then=imm2)
        dp[2].enable_alu(AluOp.SELECT, D0, D4)
        # st3: e0o = IS_EQ(src1, c0); capture selE -> lane0
        dp[3].enable_alu(AluOp.IS_EQ, D1, D2)
        dp[3].enable_delay_from_src(DelayInp.PREV_ALU_OUT, 0)
        # st4: c1o = IS_EQ(src1, c1const); capture e0o -> lane1
        dp[4].enable_alu(AluOp.IS_EQ, D1, D3)
        dp[4].enable_delay_from_src(DelayInp.PREV_ALU_OUT, 1)
        # st5: selO = SELECT(cond=c1o via PREV, else=e0o, then=imm2)
        dp[5].enable_alu(AluOp.SELECT, D1, D4)
        # st6: pair = selO + selE
        dp[6].enable_alu(AluOp.ADD, PREV, D0)
        # st7: accumulator
        if kind == "seed":
            dp[7].enable_alu(AluOp.BYPASS, D5, D5)
        elif kind == "step":
            dp[7].enable_alu(AluOp.ADD, D5, PREV)   # acc <- 0 + pair
        else:
            dp[7].enable_alu(AluOp.ADD, CURR, PREV)
        return dp

    inp = [InpSel.ZERO, InpSel.SRC_0, InpSel.SRC_1, InpSel.CONST_0,
           InpSel.CONST_1, InpSel.CONST_2, InpSel.ZERO, InpSel.ZERO]
    inp_enable = [DISABLE, ENABLE, ENABLE, ENABLE, ENABLE, ENABLE, ENABLE,
                  DISABLE]

    def mk(kind: str) -> UopConfig:
        out = {o: OutSel.ALU_OUT for o in OutPath}
        out_enable = {o: DISABLE for o in OutPath}
        if kind != "seed":
            out_enable[OutPath.WR0_LO] = ENABLE
        if kind == "seed":
            trigger = (Trigger.COUNT, Trigger.NONE, Trigger.NONE)
            next_uop, repeat = (1, 0, 0), 1
        elif kind == "steady":
            if SEGMENTED:
                trigger = (Trigger.SRC_TENSOR_DONE, Trigger.SUB_DIM_DONE,
                           Trigger.NONE)
                next_uop, repeat = (0, 2, 0), 0
            else:
                trigger = (Trigger.SRC_TENSOR_DONE, Trigger.NONE,
                           Trigger.NONE)
                next_uop, repeat = (0, 0, 0), 0
        else:  # step
            trigger = (Trigger.SRC_TENSOR_DONE, Trigger.SUB_DIM_DONE,
                       Trigger.COUNT)
            next_uop, repeat = (0, 2, 1), 1
        return UopConfig(
            datapath_config=dp_stages(kind),
            inp=list(inp),
            inp_enable=list(inp_enable),
            out=out,
            out_enable=out_enable,
            accum_enabled=DISABLE,
            require_inp0=0 if kind == "seed" else 1,
            require_inp1=0 if kind == "seed" else 1,
            trigger=trigger,
            next_uop=next_uop,
            repeat_count=repeat,
        )

    if SEGMENTED:
        return [mk("seed"), mk("steady"), mk("step")]
    return [mk("seed"), mk("steady")]


def _register_ops():
    import concourse.dve_ops as dve_ops
    from concourse.dve_spec import (
        Spec, Src0, Src1, C0, C1, C2, scan, AluOp, eq, select,
    )
    from concourse.dve_uop import DveOpSpec

    def reg(name, spec, uops=None, subdim=False):
        """Register op; if `uops` given, inject the hand-built program via
        the compile cache (bypassing lower())."""
        for op in dve_ops.OPS:
            if op.name == name:
                return op
        row = dve_ops._CUSTOM_DVE_ROW_BASE + len(dve_ops.OPS)
        assert row < 0x20, "out of custom-DVE opcode rows"
        shas = {}
        for ver in ("v3", "v4"):
            if uops is not None:
                s = DveOpSpec(name=name, opcode=row, uops=uops, rd1_en=True)
            else:
                from concourse.dve_spec import lower, _has_src1
                s = DveOpSpec(name=name, opcode=row, uops=lower(spec, ver=ver),
                              rd1_en=_has_src1(spec))
            shas[ver] = s.sha(ver)
            if uops is not None:
                dve_ops._COMPILE_CACHE[(name, ver)] = s
        op = dve_ops.DveOp(name, spec, subdim=subdim, uops_sha=shas)
        dve_ops.OPS.append(op)
        dve_ops.CUSTOM_DVE_SPECS[name] = spec
        dve_ops._SUB_OPCODE_FOR_NAME[name] = row
        return op

    def _ref_pair2(in0, in1, s0, s1, imm2):
        a = in0.astype(np.float64)
        b = in1.astype(np.float64)
        d = ((a == s0) * 1.0 + (a == s1) * imm2
             + (b == s0) * 1.0 + (b == s1) * imm2)
        if SEGMENTED:
            # prefix scan resets at each boundary of the innermost dim
            return np.cumsum(d, axis=-1).astype(np.float32)
        return np.cumsum(d.reshape(d.shape[0], -1), axis=1).astype(np.float32)

    d0 = select(eq(Src0, C1), C2, eq(Src0, C0))
    d1 = select(eq(Src1, C1), C2, eq(Src1, C0))
    pair2 = reg("ENT_PAIR2_SCAN",
                Spec(body=scan(AluOp.ADD, d0 + d1), reference=_ref_pair2),
                uops=_build_pair2_uops(), subdim=SEGMENTED)
    return pair2


def _build_nc(repeat=1):
    from contextlib import ExitStack, nullcontext
    import concourse.bacc as bacc
    import concourse.mybir as mybir
    from concourse.tile import TileContext

    PAIR2 = _register_ops()
    dt = mybir.dt
    Alu = mybir.AluOpType
    Act = mybir.ActivationFunctionType

    nc = bacc.Bacc()
    xe = nc.dram_tensor("xe", [ROWS_PC, PAIRS], dt.float16, kind="ExternalInput")
    xo = nc.dram_tensor("xo", [ROWS_PC, PAIRS], dt.float16, kind="ExternalInput")
    y = nc.dram_tensor("y", [ROWS_PC, 1], dt.float32, kind="ExternalOutput")

    # partition p owns rows [p*RPP, (p+1)*RPP); chunk c covers RC rows
    xev = xe[:].rearrange("(p c r) q -> p c (r q)", p=P, c=NCHUNK)
    xov = xo[:].rearrange("(p c r) q -> p c (r q)", p=P, c=NCHUNK)
    yv = y[:].rearrange("(p c r) o -> p c (r o)", p=P, c=NCHUNK)

    NF = RC * PAIRS            # 2048 pairs per partition per chunk
    NA = RC * NL               # 1280 limb accumulators per partition per chunk
    inv_sp = float(1.0 / S_PRIME)

    with TileContext(nc) as tc:
        with ExitStack() as ctx:
            xpool = ctx.enter_context(tc.tile_pool(name="xp", bufs=3))
            ppool = ctx.enter_context(tc.tile_pool(name="pp", bufs=4))
            apool = ctx.enter_context(tc.tile_pool(name="ap", bufs=2))
            dpool = ctx.enter_context(tc.tile_pool(name="dp", bufs=2))
            epool = ctx.enter_context(tc.tile_pool(name="ep", bufs=2))
            singles = ctx.enter_context(tc.tile_pool(name="sg", bufs=1))

            t_eps = singles.tile([P, 1], dt.float32)
            nc.vector.memset(t_eps[:], EPS)
            t_inv = singles.tile([P, 1], dt.float32)
            nc.vector.memset(t_inv[:], inv_sp)
            t_mg = singles.tile([P, 1], dt.float32)
            nc.vector.memset(t_mg[:], float(MAGIC - 0.25))
            t_nmg = singles.tile([P, 1], dt.float32)
            nc.vector.memset(t_nmg[:], float(-MAGIC))
            t_invw = singles.tile([P, 1], dt.float32)
            nc.vector.memset(t_invw[:], float(1.0 / W))
            t_one = singles.tile([P, 1], dt.float32)
            nc.vector.memset(t_one[:], 1.0)

            repctx = tc.For_i(0, repeat, 1) if repeat > 1 else nullcontext()
            with repctx:
              for c in range(NCHUNK):
                xet = xpool.tile([P, NF], dt.float16, tag="xe")
                xot = xpool.tile([P, NF], dt.float16, tag="xo")
                nc.sync.dma_start(out=xet[:], in_=xev[:, c, :])
                nc.sync.dma_start(out=xot[:], in_=xov[:, c, :])

                Ab = apool.tile([P, RC, NL], dt.float32, tag="A")

                for g in range(NL):
                    pref = ppool.tile([P, RC, PAIRS], dt.float32, tag="pref")
                    if SEGMENTED:
                        nc.vector._custom_dve(
                            PAIR2,
                            out=pref[:],
                            in0=xet[:].rearrange("p (r q) -> p r q", q=PAIRS),
                            in1=xot[:],
                            s0=float(2 * g), s1=float(2 * g + 1), imm2=W)
                        # scan resets each row: prefix at q=PAIRS-1 IS the sum
                        nc.scalar.copy(Ab[:, :, g], pref[:, :, PAIRS - 1])
                    else:
                        nc.vector._custom_dve(
                            PAIR2,
                            out=pref[:].rearrange("p r q -> p (r q)"),
                            in0=xet[:], in1=xot[:],
                            s0=float(2 * g), s1=float(2 * g + 1), imm2=W)
                        # row sums from prefix boundary differences
                        nc.scalar.copy(Ab[:, 0:1, g], pref[:, 0:1, PAIRS - 1])
                        nc.gpsimd.tensor_tensor(
                            out=Ab[:, 1:, g],
                            in0=pref[:, 1:, PAIRS - 1],
                            in1=pref[:, :-1, PAIRS - 1],
                            op=Alu.subtract)

                Af = Ab[:].rearrange("p r g -> p (r g)")       # [P, NA]
                # c1 = rint(A/128 - 0.25) via exact magic rounding on ACT
                t1 = dpool.tile([P, NA], dt.float32, tag="t1")
                c1 = dpool.tile([P, NA], dt.float32, tag="c1")
                nc.scalar.activation(t1[:], Af, Act.Relu,
                                     bias=t_mg[:], scale=t_invw[:])
                nc.scalar.activation(c1[:], t1[:], Act.Relu, bias=t_nmg[:],
                                     scale=t_one[:])
                # c0 = A - 128*c1
                c0 = dpool.tile([P, NA], dt.float32, tag="c0")
                nc.vector.scalar_tensor_tensor(
                    out=c0[:], in0=c1[:], scalar=-W, in1=Af,
                    op0=Alu.mult, op1=Alu.add)

                # u_i = log(c_i/S' + eps)
                u0 = dpool.tile([P, NA], dt.float32, tag="u0")
                u1 = dpool.tile([P, NA], dt.float32, tag="u1")
                nc.scalar.activation(u0[:], c0[:], Act.Ln,
                                     bias=t_eps[:], scale=t_inv[:])
                nc.scalar.activation(u1[:], c1[:], Act.Ln,
                                     bias=t_eps[:], scale=t_inv[:])

                # per-row entropy: e = sum_g c0*u0 + c1*u1 over NL lanes
                m0 = dpool.tile([P, RC, NL], dt.float32, tag="m0")
                m1 = dpool.tile([P, RC, NL], dt.float32, tag="m1")
                nc.gpsimd.tensor_tensor(
                    out=m0[:].rearrange("p r g -> p (r g)"),
                    in0=c0[:], in1=u0[:], op=Alu.mult)
                nc.gpsimd.tensor_tensor(
                    out=m1[:].rearrange("p r g -> p (r g)"),
                    in0=c1[:], in1=u1[:], op=Alu.mult)
                nc.gpsimd.tensor_tensor(
                    out=m0[:].rearrange("p r g -> p (r g)"),
                    in0=m0[:].rearrange("p r g -> p (r g)"),
                    in1=m1[:].rearrange("p r g -> p (r g)"), op=Alu.add)
                eout = epool.tile([P, RC], dt.float32, tag="eout")
                nc.vector.tensor_reduce(
                    out=eout[:], in_=m0[:], axis=mybir.AxisListType.X,
                    op=Alu.add)
                nc.vector.tensor_scalar_mul(eout[:], eout[:],
                                            float(-1.0 / S_PRIME))
                nc.sync.dma_start(out=yv[:, c, :], in_=eout[:])

    nc.finalize()
    return nc


def _build_runner(repeat=1):
    """Cached jitted 8-core runner (modeled on bass2jax.run_bass_via_pjrt,
    but reusing one jitted executable across calls)."""
    import jax
    from jax.sharding import Mesh, PartitionSpec
    from jax.experimental.shard_map import shard_map
    import concourse.bass2jax as b2j

    nc = _build_nc(repeat=repeat)
    b2j.install_neuronx_cc_hook()

    import concourse.mybir as mybir
    partition_name = (nc.partition_id_tensor.name
                      if nc.partition_id_tensor else None)
    in_names, out_names, out_avals, zero_outs = [], [], [], []
    for alloc in nc.m.functions[0].allocations:
        if not isinstance(alloc, mybir.MemoryLocationSet):
            continue
        name = alloc.memorylocations[0].name
        if alloc.kind == "ExternalInput":
            if name != partition_name:
                in_names.append(name)
        elif alloc.kind == "ExternalOutput":
            shape = tuple(alloc.tensor_shape)
            dtype = mybir.dt.np(alloc.dtype)
            out_names.append(name)
            out_avals.append(jax.core.ShapedArray(shape, dtype))
            zero_outs.append(np.zeros(shape, dtype))
    n_params = len(in_names)
    all_in_names = in_names + out_names
    if partition_name is not None:
        all_in_names = all_in_names + [partition_name]

    def _body(*args):
        operands = list(args)
        if partition_name is not None:
            operands.append(b2j.partition_id_tensor())
        outs = b2j._bass_exec_p.bind(
            *operands,
            out_avals=tuple(out_avals),
            in_names=tuple(all_in_names),
            out_names=tuple(out_names),
            lowering_input_output_aliases=(),
            sim_require_finite=True,
            sim_require_nnan=True,
            nc=nc,
        )
        return tuple(outs)

    devices = jax.devices()[:NCORES]
    mesh = Mesh(np.asarray(devices), ("core",))
    n_outs = len(out_avals)
    sharded = jax.jit(
        shard_map(_body, mesh=mesh,
                  in_specs=(PartitionSpec("core"),) * (n_params + n_outs),
                  out_specs=(PartitionSpec("core"),) * n_outs,
                  check_rep=False),
        donate_argnums=tuple(range(n_params, n_params + n_outs)),
        keep_unused=True,
    )

    def run(*ins) -> np.ndarray:
        zeros = [np.zeros((NCORES * z.shape[0], *z.shape[1:]), z.dtype)
                 for z in zero_outs]
        out = sharded(*ins, *zeros)
        return np.asarray(out[0])

    run.sharded = sharded
    run.zero_outs = zero_outs
    run.mesh = mesh
    run.in_names = in_names
    return run


def prep_inputs(x: np.ndarray):
    """Full input -> device-input tuple (even fp16, odd fp16)."""
    x = np.asarray(x)
    assert x.shape == (B, L), x.shape
    xe = np.ascontiguousarray(x[:, 0::2]).astype(np.float16)
    xo = np.ascontiguousarray(x[:, 1::2]).astype(np.float16)
    return xe, xo


def kernel(x: np.ndarray) -> np.ndarray:
    global _RUNNER
    ins = prep_inputs(x)
    if _RUNNER is None:
        _RUNNER = _build_runner()
    try:
        out = _RUNNER(*ins)
    except Exception:
        # transient device hiccups (NRT exec-unit resets) have been observed
        # once on this fabric; one retry after a short pause recovers.
        import time
        time.sleep(20.0)
        out = _RUNNER(*ins)
    return out.reshape(B, 1).astype(np.float32)


if __name__ == "__main__":
    rng = np.random.default_rng(0)
    xa = rng.integers(0, VOCAB, size=(B, L)).astype(np.int32)
    out = kernel(x=xa)
    cnt = np.zeros((B, VOCAB), np.float64)
    for v in range(VOCAB):
        cnt[:, v] = (xa == v).sum(1)
    p = cnt / S_PRIME
    ref = -(p * np.log(p + EPS)).sum(1, keepdims=True)
    err = np.abs(out - ref).max()
    rel = err / np.abs(ref).max()
    print("selfcheck max abs err:", err, "rel:", rel)


# revision 27
# speedup vs baseline: 1.0239x; 1.0239x over previous
"""Trainium2 Bass kernel for nn_EntropyCalculator (per-row histogram entropy).

x: [262144, 64] ints in [0, 40). Output: [262144, 1] float32 per-row entropy
of the value histogram: -sum_v p_v*log(p_v + 1e-8), p = c/(64+1e-8).

Strategy (per core, pure data parallel over 8 cores):
  The 40-bin histogram is computed with 20 "pair limb" passes on the DVE.
  Pass g consumes TWO elements per cycle (even/odd streams on the two SBUF
  read ports) and accumulates d(a) + d(b) into a running prefix, where
  d(x) = 1 if x == 2g, 128 if x == 2g+1, else 0 (an 8-stage hand-placed
  uop program: IS_EQ, IS_EQ, SELECT per element + pair-add + scan).
  Per-row sums are recovered by strided differences of the prefix at row
  boundaries; digits c0 = count(2g), c1 = count(2g+1) are split with an
  exact magic-number rint on the ACT engine, and the entropy tail
  (Ln + c*u dot) runs on ACT/GPSIMD/DVE.

  Exactness: pair values are integers <= 256; chunk prefixes < 2^24, so
  every fp32 partial sum is exact.  A = c0 + 128*c1 with c0, c1 <= 64;
  c1 = rint(A/128 - 0.25) exactly (c0/128 - 0.25 in [-0.25, 0.25]).
"""

import numpy as np

VOCAB = 40
L = 64
B = 262144
NCORES = 8
ROWS_PC = B // NCORES          # 32768 rows per core
P = 128                        # SBUF partitions
RPP = ROWS_PC // P             # 256 rows per partition
RC = 64                        # rows per partition per chunk
NCHUNK = RPP // RC             # 4 chunks
PAIRS = L // 2                 # 32 element-pairs per row
NL = VOCAB // 2                # 20 pair limbs
W = 128.0                      # digit weight for the odd bin of a limb
EPS = 1e-8
S_PRIME = 64.0 + EPS
MAGIC = 8388608.0              # 2^23
SEGMENTED = False              # scan resets per row (measured slower: uop
                               # transitions cost ~3 cy/row) vs boundary diffs

import os as _os
RC = int(_os.environ.get("ENT_RC", RC))
NCHUNK = RPP // RC
MAPONLY = bool(int(_os.environ.get("ENT_MAPONLY", "0")))  # timing experiment

_RUNNER = None


def _build_pair2_uops():
    """Hand-placed 8-stage uop program for the PAIR2 op:

        out[k] = scan_add( d(in0[k]) + d(in1[k]) ),
        d(x) = select(x == s1, imm2, x == s0)

    Two elements per cycle (both read ports), fused map+pair-add+scan.
    The stock scheduler cannot place this in 8 stages (select-cond shims);
    hand placement puts each SELECT's cond at stage-1 via PREV_ALU_OUT.
    """
    from concourse.dve_uop import (
        UopConfig, UopDpConfig, AluInp, AluOp, DelayInp, InpSel, OutPath,
        OutSel, Trigger, ENABLE, DISABLE,
    )

    D0, D1, D2, D3, D4, D5 = (
        AluInp.PREV_DELAY_0, AluInp.PREV_DELAY_1, AluInp.PREV_DELAY_2,
        AluInp.PREV_DELAY_3, AluInp.PREV_DELAY_4, AluInp.PREV_DELAY_5,
    )
    PREV = AluInp.PREV_ALU_OUT
    CURR = AluInp.CURR_ALU_OUT

    def dp_stages(kind: str):
        # kind: "seed" (acc <- 0), "steady" (acc += pair),
        #       "step" (acc <- 0 + pair; fires on row boundary)
        dp = [UopDpConfig() for _ in range(8)]
        for st in range(8):
            dp[st].pass_through_delay(0, 1, 2, 3, 4, 5)
        # st0: e0  = IS_EQ(src0, c0)
        dp[0].enable_alu(AluOp.IS_EQ, D0, D2)
        # st1: c1  = IS_EQ(src0, c1const); capture e0 -> lane0
        dp[1].enable_alu(AluOp.IS_EQ, D0, D3)
        dp[1].enable_delay_from_src(DelayInp.PREV_ALU_OUT, 0)
        # st2: selE = SELECT(cond=c1 via PREV, else=e0, then=imm2)
        dp[2].enable_alu(AluOp.SELECT, D0, D4)
        # st3: e0o = IS_EQ(src1, c0); capture selE -> lane0
        dp[3].enable_alu(AluOp.IS_EQ, D1, D2)
        dp[3].enable_delay_from_src(DelayInp.PREV_ALU_OUT, 0)
        # st4: c1o = IS_EQ(src1, c1const); capture e0o -> lane1
        dp[4].enable_alu(AluOp.IS_EQ, D1, D3)
        dp[4].enable_delay_from_src(DelayInp.PREV_ALU_OUT, 1)
        # st5: selO = SELECT(cond=c1o via PREV, else=e0o, then=imm2)
        dp[5].enable_alu(AluOp.SELECT, D1, D4)
        # st6: pair = selO + selE
        dp[6].enable_alu(AluOp.ADD, PREV, D0)
        # st7: accumulator
        if kind == "seed":
            dp[7].enable_alu(AluOp.BYPASS, D5, D5)
        elif kind == "step":
            dp[7].enable_alu(AluOp.ADD, D5, PREV)   # acc <- 0 + pair
        else:
            dp[7].enable_alu(AluOp.ADD, CURR, PREV)
        return dp

    inp = [InpSel.ZERO, InpSel.SRC_0, InpSel.SRC_1, InpSel.CONST_0,
           InpSel.CONST_1, InpSel.CONST_2, InpSel.ZERO, InpSel.ZERO]
    inp_enable = [DISABLE, ENABLE, ENABLE, ENABLE, ENABLE, ENABLE, ENABLE,
                  DISABLE]

    def mk(kind: str) -> UopConfig:
        out = {o: OutSel.ALU_OUT for o in OutPath}
        out_enable = {o: DISABLE for o in OutPath}
        if kind != "seed":
            out_enable[OutPath.WR0_LO] = ENABLE
        if kind == "seed":
            trigger = (Trigger.COUNT, Trigger.NONE, Trigger.NONE)
            next_uop, repeat = (1, 0, 0), 1
        elif kind == "steady":
            if SEGMENTED:
                trigger = (Trigger.SRC_TENSOR_DONE, Trigger.SUB_DIM_DONE,
                           Trigger.NONE)
                next_uop, repeat = (0, 2, 0), 0
            else:
                trigger = (Trigger.SRC_TENSOR_DONE, Trigger.NONE,
                           Trigger.NONE)
                next_uop, repeat = (0, 0, 0), 0
        else:  # step
            trigger = (Trigger.SRC_TENSOR_DONE, Trigger.SUB_DIM_DONE,
                       Trigger.COUNT)
            next_uop, repeat = (0, 2, 1), 1
        return UopConfig(
            datapath_config=dp_stages(kind),
            inp=list(inp),
            inp_enable=list(inp_enable),
            out=out,
            out_enable=out_enable,
            accum_enabled=DISABLE,
            require_inp0=0 if kind == "seed" else 1,
            require_inp1=0 if kind == "seed" else 1,
            trigger=trigger,
            next_uop=next_uop,
            repeat_count=repeat,
        )

    if SEGMENTED:
        return [mk("seed"), mk("steady"), mk("step")]
    return [mk("seed"), mk("steady")]


def _register_ops():
    import concourse.dve_ops as dve_ops
    from concourse.dve_spec import (
        Spec, Src0, Src1, C0, C1, C2, scan, AluOp, eq, select,
    )
    from concourse.dve_uop import DveOpSpec

    def reg(name, spec, uops=None, subdim=False):
        """Register op; if `uops` given, inject the hand-built program via
        the compile cache (bypassing lower())."""
        for op in dve_ops.OPS:
            if op.name == name:
                return op
        row = dve_ops._CUSTOM_DVE_ROW_BASE + len(dve_ops.OPS)
        assert row < 0x20, "out of custom-DVE opcode rows"
        shas = {}
        for ver in ("v3", "v4"):
            if uops is not None:
                s = DveOpSpec(name=name, opcode=row, uops=uops, rd1_en=True)
            else:
                from concourse.dve_spec import lower, _has_src1
                s = DveOpSpec(name=name, opcode=row, uops=lower(spec, ver=ver),
                              rd1_en=_has_src1(spec))
            shas[ver] = s.sha(ver)
            if uops is not None:
                dve_ops._COMPILE_CACHE[(name, ver)] = s
        op = dve_ops.DveOp(name, spec, subdim=subdim, uops_sha=shas)
        dve_ops.OPS.append(op)
        dve_ops.CUSTOM_DVE_SPECS[name] = spec
        dve_ops._SUB_OPCODE_FOR_NAME[name] = row
        return op

    def _ref_pair2(in0, in1, s0, s1, imm2):
        a = in0.astype(np.float64)
        b = in1.astype(np.float64)
        d = ((a == s0) * 1.0 + (a == s1) * imm2
             + (b == s0) * 1.0 + (b == s1) * imm2)
        if SEGMENTED:
            # prefix scan resets at each boundary of the innermost dim
            return np.cumsum(d, axis=-1).astype(np.float32)
        return np.cumsum(d.reshape(d.shape[0], -1), axis=1).astype(np.float32)

    d0 = select(eq(Src0, C1), C2, eq(Src0, C0))
    d1 = select(eq(Src1, C1), C2, eq(Src1, C0))
    pair2 = reg("ENT_PAIR2_SCAN",
                Spec(body=scan(AluOp.ADD, d0 + d1), reference=_ref_pair2),
                uops=_build_pair2_uops(), subdim=SEGMENTED)
    return pair2


def _build_nc(repeat=1):
    from contextlib import ExitStack, nullcontext
    import concourse.bacc as bacc
    import concourse.mybir as mybir
    from concourse.tile import TileContext

    PAIR2 = _register_ops()
    dt = mybir.dt
    Alu = mybir.AluOpType
    Act = mybir.ActivationFunctionType

    nc = bacc.Bacc()
    xe = nc.dram_tensor("xe", [ROWS_PC, PAIRS], dt.float16, kind="ExternalInput")
    xo = nc.dram_tensor("xo", [ROWS_PC, PAIRS], dt.float16, kind="ExternalInput")
    y = nc.dram_tensor("y", [ROWS_PC, 1], dt.float32, kind="ExternalOutput")

    # partition p owns rows [p*RPP, (p+1)*RPP); chunk c covers RC rows
    xev = xe[:].rearrange("(p c r) q -> p c (r q)", p=P, c=NCHUNK)
    xov = xo[:].rearrange("(p c r) q -> p c (r q)", p=P, c=NCHUNK)
    yv = y[:].rearrange("(p c r) o -> p c (r o)", p=P, c=NCHUNK)

    NF = RC * PAIRS            # 2048 pairs per partition per chunk
    NA = RC * NL               # 1280 limb accumulators per partition per chunk
    inv_sp = float(1.0 / S_PRIME)

    with TileContext(nc) as tc:
        with ExitStack() as ctx:
            xpool = ctx.enter_context(tc.tile_pool(name="xp", bufs=3))
            ppool = ctx.enter_context(tc.tile_pool(name="pp", bufs=4))
            apool = ctx.enter_context(tc.tile_pool(name="ap", bufs=2))
            dpool = ctx.enter_context(tc.tile_pool(name="dp", bufs=2))
            epool = ctx.enter_context(tc.tile_pool(name="ep", bufs=2))
            singles = ctx.enter_context(tc.tile_pool(name="sg", bufs=1))

            t_eps = singles.tile([P, 1], dt.float32)
            nc.vector.memset(t_eps[:], EPS)
            t_inv = singles.tile([P, 1], dt.float32)
            nc.vector.memset(t_inv[:], inv_sp)
            t_mg = singles.tile([P, 1], dt.float32)
            nc.vector.memset(t_mg[:], float(MAGIC - 0.25))
            t_nmg = singles.tile([P, 1], dt.float32)
            nc.vector.memset(t_nmg[:], float(-MAGIC))
            t_invw = singles.tile([P, 1], dt.float32)
            nc.vector.memset(t_invw[:], float(1.0 / W))
            t_one = singles.tile([P, 1], dt.float32)
            nc.vector.memset(t_one[:], 1.0)
            t_mw = singles.tile([P, 1], dt.float32)
            nc.vector.memset(t_mw[:], float(-W))
            t_zero = singles.tile([P, 1], dt.float32)
            nc.vector.memset(t_zero[:], 0.0)

            def flush_back(item):
                cc, m0c = item
                eout = epool.tile([P, RC], dt.float32, tag="eout")
                nc.vector.tensor_reduce(
                    out=eout[:], in_=m0c[:], axis=mybir.AxisListType.X,
                    op=Alu.add)
                nc.vector.tensor_scalar_mul(eout[:], eout[:],
                                            float(-1.0 / S_PRIME))
                nc.sync.dma_start(out=yv[:, cc, :], in_=eout[:])

            repctx = tc.For_i(0, repeat, 1) if repeat > 1 else nullcontext()
            with repctx:
              pending = []
              for c in range(NCHUNK):
                xet = xpool.tile([P, NF], dt.float16, tag="xe")
                xot = xpool.tile([P, NF], dt.float16, tag="xo")
                nc.sync.dma_start(out=xet[:], in_=xev[:, c, :])
                nc.sync.dma_start(out=xot[:], in_=xov[:, c, :])

                Ab = apool.tile([P, RC, NL], dt.float32, tag="A")

                if MAPONLY:
                    # timing experiment: the 20 map scans + dummy output only
                    for g in range(NL):
                        pref = ppool.tile([P, RC, PAIRS], dt.float32,
                                          tag="pref")
                        nc.vector._custom_dve(
                            PAIR2,
                            out=pref[:].rearrange("p r q -> p (r q)"),
                            in0=xet[:], in1=xot[:],
                            s0=float(2 * g), s1=float(2 * g + 1), imm2=W)
                    eout = epool.tile([P, RC], dt.float32, tag="eout")
                    nc.scalar.copy(eout[:], pref[:, :, PAIRS - 1])
                    nc.sync.dma_start(out=yv[:, c, :], in_=eout[:])
                    continue

                for g in range(NL):
                    pref = ppool.tile([P, RC, PAIRS], dt.float32, tag="pref")
                    if SEGMENTED:
                        nc.vector._custom_dve(
                            PAIR2,
                            out=pref[:],
                            in0=xet[:].rearrange("p (r q) -> p r q", q=PAIRS),
                            in1=xot[:],
                            s0=float(2 * g), s1=float(2 * g + 1), imm2=W)
                        # scan resets each row: prefix at q=PAIRS-1 IS the sum
                        nc.scalar.copy(Ab[:, :, g], pref[:, :, PAIRS - 1])
                    else:
                        nc.vector._custom_dve(
                            PAIR2,
                            out=pref[:].rearrange("p r q -> p (r q)"),
                            in0=xet[:], in1=xot[:],
                            s0=float(2 * g), s1=float(2 * g + 1), imm2=W)
                        # row sums from prefix boundary differences
                        nc.scalar.copy(Ab[:, 0:1, g], pref[:, 0:1, PAIRS - 1])
                        nc.gpsimd.tensor_tensor(
                            out=Ab[:, 1:, g],
                            in0=pref[:, 1:, PAIRS - 1],
                            in1=pref[:, :-1, PAIRS - 1],
                            op=Alu.subtract)

                Af = Ab[:].rearrange("p r g -> p (r g)")       # [P, NA]
                # c1 = rint(A/128 - 0.25) via exact magic rounding on ACT
                t1 = dpool.tile([P, NA], dt.float32, tag="t1")
                c1 = dpool.tile([P, NA], dt.float32, tag="c1")
                nc.scalar.activation(t1[:], Af, Act.Relu,
                                     bias=t_mg[:], scale=t_invw[:])
                nc.scalar.activation(c1[:], t1[:], Act.Relu, bias=t_nmg[:],
                                     scale=t_one[:])
                # c0 = A - 128*c1 (ACT scale + gpsimd add; keep DVE free)
                c1m = dpool.tile([P, NA], dt.float32, tag="c1m")
                nc.scalar.activation(c1m[:], c1[:], Act.Copy, bias=0.0,
                                     scale=float(-W))
                c0 = dpool.tile([P, NA], dt.float32, tag="c0")
                nc.gpsimd.tensor_tensor(out=c0[:], in0=Af, in1=c1m[:],
                                        op=Alu.add)

                # u_i = log(c_i/S' + eps)
                u0 = dpool.tile([P, NA], dt.float32, tag="u0")
                u1 = dpool.tile([P, NA], dt.float32, tag="u1")
                nc.scalar.activation(u0[:], c0[:], Act.Ln,
                                     bias=t_eps[:], scale=t_inv[:])
                nc.scalar.activation(u1[:], c1[:], Act.Ln,
                                     bias=t_eps[:], scale=t_inv[:])

                # per-row entropy: e = sum_g c0*u0 + c1*u1 over NL lanes
                m0 = dpool.tile([P, RC, NL], dt.float32, tag="m0")
                m1 = dpool.tile([P, RC, NL], dt.float32, tag="m1")
                nc.gpsimd.tensor_tensor(
                    out=m0[:].rearrange("p r g -> p (r g)"),
                    in0=c0[:], in1=u0[:], op=Alu.mult)
                nc.gpsimd.tensor_tensor(
                    out=m1[:].rearrange("p r g -> p (r g)"),
                    in0=c1[:], in1=u1[:], op=Alu.mult)
                nc.gpsimd.tensor_tensor(
                    out=m0[:].rearrange("p r g -> p (r g)"),
                    in0=m0[:].rearrange("p r g -> p (r g)"),
                    in1=m1[:].rearrange("p r g -> p (r g)"), op=Alu.add)
                pending.append((c, m0))
                # Defer this chunk's DVE decode ops (reduce + scale) until
                # after the NEXT chunk's map stream: the DVE queue is strict
                # FIFO, so issuing them here would stall the DVE on the
                # ACT/gpsimd decode chain.
                if len(pending) > 1:
                    flush_back(pending.pop(0))
              for item in pending:
                flush_back(item)
              pending.clear()

    nc.finalize()
    return nc


def _build_runner(repeat=1):
    """Cached jitted 8-core runner (modeled on bass2jax.run_bass_via_pjrt,
    but reusing one jitted executable across calls)."""
    import jax
    from jax.sharding import Mesh, PartitionSpec
    from jax.experimental.shard_map import shard_map
    import concourse.bass2jax as b2j

    nc = _build_nc(repeat=repeat)
    b2j.install_neuronx_cc_hook()

    import concourse.mybir as mybir
    partition_name = (nc.partition_id_tensor.name
                      if nc.partition_id_tensor else None)
    in_names, out_names, out_avals, zero_outs = [], [], [], []
    for alloc in nc.m.functions[0].allocations:
        if not isinstance(alloc, mybir.MemoryLocationSet):
            continue
        name = alloc.memorylocations[0].name
        if alloc.kind == "ExternalInput":
            if name != partition_name:
                in_names.append(name)
        elif alloc.kind == "ExternalOutput":
            shape = tuple(alloc.tensor_shape)
            dtype = mybir.dt.np(alloc.dtype)
            out_names.append(name)
            out_avals.append(jax.core.ShapedArray(shape, dtype))
            zero_outs.append(np.zeros(shape, dtype))
    n_params = len(in_names)
    all_in_names = in_names + out_names
    if partition_name is not None:
        all_in_names = all_in_names + [partition_name]

    def _body(*args):
        operands = list(args)
        if partition_name is not None:
            operands.append(b2j.partition_id_tensor())
        outs = b2j._bass_exec_p.bind(
            *operands,
            out_avals=tuple(out_avals),
            in_names=tuple(all_in_names),
            out_names=tuple(out_names),
            lowering_input_output_aliases=(),
            sim_require_finite=True,
            sim_require_nnan=True,
            nc=nc,
        )
        return tuple(outs)

    devices = jax.devices()[:NCORES]
    mesh = Mesh(np.asarray(devices), ("core",))
    n_outs = len(out_avals)
    sharded = jax.jit(
        shard_map(_body, mesh=mesh,
                  in_specs=(PartitionSpec("core"),) * (n_params + n_outs),
                  out_specs=(PartitionSpec("core"),) * n_outs,
                  check_rep=False),
        donate_argnums=tuple(range(n_params, n_params + n_outs)),
        keep_unused=True,
    )

    def run(*ins) -> np.ndarray:
        zeros = [np.zeros((NCORES * z.shape[0], *z.shape[1:]), z.dtype)
                 for z in zero_outs]
        out = sharded(*ins, *zeros)
        return np.asarray(out[0])

    run.sharded = sharded
    run.zero_outs = zero_outs
    run.mesh = mesh
    run.in_names = in_names
    return run


def prep_inputs(x: np.ndarray):
    """Full input -> device-input tuple (even fp16, odd fp16)."""
    x = np.asarray(x)
    assert x.shape == (B, L), x.shape
    xe = np.ascontiguousarray(x[:, 0::2]).astype(np.float16)
    xo = np.ascontiguousarray(x[:, 1::2]).astype(np.float16)
    return xe, xo


def kernel(x: np.ndarray) -> np.ndarray:
    global _RUNNER
    ins = prep_inputs(x)
    if _RUNNER is None:
        _RUNNER = _build_runner()
    try:
        out = _RUNNER(*ins)
    except Exception:
        # transient device hiccups (NRT exec-unit resets) have been observed
        # once on this fabric; one retry after a short pause recovers.
        import time
        time.sleep(20.0)
        out = _RUNNER(*ins)
    return out.reshape(B, 1).astype(np.float32)


if __name__ == "__main__":
    rng = np.random.default_rng(0)
    xa = rng.integers(0, VOCAB, size=(B, L)).astype(np.int32)
    out = kernel(x=xa)
    cnt = np.zeros((B, VOCAB), np.float64)
    for v in range(VOCAB):
        cnt[:, v] = (xa == v).sum(1)
    p = cnt / S_PRIME
    ref = -(p * np.log(p + EPS)).sum(1, keepdims=True)
    err = np.abs(out - ref).max()
    rel = err / np.abs(ref).max()
    print("selfcheck max abs err:", err, "rel:", rel)


# revision 30
# speedup vs baseline: 1.0691x; 1.0441x over previous
"""Trainium2 Bass kernel for nn_EntropyCalculator (per-row histogram entropy).

x: [262144, 64] ints in [0, 40). Output: [262144, 1] float32 per-row entropy
of the value histogram: -sum_v p_v*log(p_v + 1e-8), p = c/(64+1e-8).

Strategy (per core, pure data parallel over 8 cores):
  The 40-bin histogram is computed with 20 "pair limb" passes on the DVE.
  Pass g consumes TWO elements per cycle (even/odd streams on the two SBUF
  read ports) and accumulates d(a) + d(b) into a running prefix, where
  d(x) = 1 if x == 2g, 128 if x == 2g+1, else 0 (an 8-stage hand-placed
  uop program: IS_EQ, IS_EQ, SELECT per element + pair-add + scan).
  Per-row sums are recovered by strided differences of the prefix at row
  boundaries; digits c0 = count(2g), c1 = count(2g+1) are split with an
  exact magic-number rint on the ACT engine, and the entropy tail
  (Ln + c*u dot) runs on ACT/GPSIMD/DVE.

  Exactness: pair values are integers <= 256; chunk prefixes < 2^24, so
  every fp32 partial sum is exact.  A = c0 + 128*c1 with c0, c1 <= 64;
  c1 = rint(A/128 - 0.25) exactly (c0/128 - 0.25 in [-0.25, 0.25]).
"""

import numpy as np

VOCAB = 40
L = 64
B = 262144
NCORES = 8
ROWS_PC = B // NCORES          # 32768 rows per core
P = 128                        # SBUF partitions
RPP = ROWS_PC // P             # 256 rows per partition
RC = 64                        # rows per partition per chunk
NCHUNK = RPP // RC             # 4 chunks
PAIRS = L // 2                 # 32 element-pairs per row
NL = VOCAB // 2                # 20 pair limbs
W = 128.0                      # digit weight for the odd bin of a limb
EPS = 1e-8
S_PRIME = 64.0 + EPS
MAGIC = 8388608.0              # 2^23
SEGMENTED = False              # scan resets per row (measured slower: uop
                               # transitions cost ~3 cy/row) vs boundary diffs

import os as _os
RC = int(_os.environ.get("ENT_RC", RC))
NCHUNK = RPP // RC
MAPONLY = bool(int(_os.environ.get("ENT_MAPONLY", "0")))  # timing experiment

_RUNNER = None


def _build_pair2_uops():
    """Hand-placed 8-stage uop program for the PAIR2 op:

        out[k] = scan_add( d(in0[k]) + d(in1[k]) ),
        d(x) = select(x == s1, imm2, x == s0)

    Two elements per cycle (both read ports), fused map+pair-add+scan.
    The stock scheduler cannot place this in 8 stages (select-cond shims);
    hand placement puts each SELECT's cond at stage-1 via PREV_ALU_OUT.
    """
    from concourse.dve_uop import (
        UopConfig, UopDpConfig, AluInp, AluOp, DelayInp, InpSel, OutPath,
        OutSel, Trigger, ENABLE, DISABLE,
    )

    D0, D1, D2, D3, D4, D5 = (
        AluInp.PREV_DELAY_0, AluInp.PREV_DELAY_1, AluInp.PREV_DELAY_2,
        AluInp.PREV_DELAY_3, AluInp.PREV_DELAY_4, AluInp.PREV_DELAY_5,
    )
    PREV = AluInp.PREV_ALU_OUT
    CURR = AluInp.CURR_ALU_OUT

    def dp_stages(kind: str):
        # kind: "seed" (acc <- 0), "steady" (acc += pair),
        #       "step" (acc <- 0 + pair; fires on row boundary)
        dp = [UopDpConfig() for _ in range(8)]
        for st in range(8):
            dp[st].pass_through_delay(0, 1, 2, 3, 4, 5)
        # st0: e0  = IS_EQ(src0, c0)
        dp[0].enable_alu(AluOp.IS_EQ, D0, D2)
        # st1: c1  = IS_EQ(src0, c1const); capture e0 -> lane0
        dp[1].enable_alu(AluOp.IS_EQ, D0, D3)
        dp[1].enable_delay_from_src(DelayInp.PREV_ALU_OUT, 0)
        # st2: selE = SELECT(cond=c1 via PREV, else=e0, then=imm2)
        dp[2].enable_alu(AluOp.SELECT, D0, D4)
        # st3: e0o = IS_EQ(src1, c0); capture selE -> lane0
        dp[3].enable_alu(AluOp.IS_EQ, D1, D2)
        dp[3].enable_delay_from_src(DelayInp.PREV_ALU_OUT, 0)
        # st4: c1o = IS_EQ(src1, c1const); capture e0o -> lane1
        dp[4].enable_alu(AluOp.IS_EQ, D1, D3)
        dp[4].enable_delay_from_src(DelayInp.PREV_ALU_OUT, 1)
        # st5: selO = SELECT(cond=c1o via PREV, else=e0o, then=imm2)
        dp[5].enable_alu(AluOp.SELECT, D1, D4)
        # st6: pair = selO + selE
        dp[6].enable_alu(AluOp.ADD, PREV, D0)
        # st7: accumulator
        if kind == "seed":
            dp[7].enable_alu(AluOp.BYPASS, D5, D5)
        elif kind == "step":
            dp[7].enable_alu(AluOp.ADD, D5, PREV)   # acc <- 0 + pair
        else:
            dp[7].enable_alu(AluOp.ADD, CURR, PREV)
        return dp

    inp = [InpSel.ZERO, InpSel.SRC_0, InpSel.SRC_1, InpSel.CONST_0,
           InpSel.CONST_1, InpSel.CONST_2, InpSel.ZERO, InpSel.ZERO]
    inp_enable = [DISABLE, ENABLE, ENABLE, ENABLE, ENABLE, ENABLE, ENABLE,
                  DISABLE]

    def mk(kind: str) -> UopConfig:
        out = {o: OutSel.ALU_OUT for o in OutPath}
        out_enable = {o: DISABLE for o in OutPath}
        if kind != "seed":
            out_enable[OutPath.WR0_LO] = ENABLE
        if kind == "seed":
            trigger = (Trigger.COUNT, Trigger.NONE, Trigger.NONE)
            next_uop, repeat = (1, 0, 0), 1
        elif kind == "steady":
            if SEGMENTED:
                trigger = (Trigger.SRC_TENSOR_DONE, Trigger.SUB_DIM_DONE,
                           Trigger.NONE)
                next_uop, repeat = (0, 2, 0), 0
            else:
                trigger = (Trigger.SRC_TENSOR_DONE, Trigger.NONE,
                           Trigger.NONE)
                next_uop, repeat = (0, 0, 0), 0
        else:  # step
            trigger = (Trigger.SRC_TENSOR_DONE, Trigger.SUB_DIM_DONE,
                       Trigger.COUNT)
            next_uop, repeat = (0, 2, 1), 1
        return UopConfig(
            datapath_config=dp_stages(kind),
            inp=list(inp),
            inp_enable=list(inp_enable),
            out=out,
            out_enable=out_enable,
            accum_enabled=DISABLE,
            require_inp0=0 if kind == "seed" else 1,
            require_inp1=0 if kind == "seed" else 1,
            trigger=trigger,
            next_uop=next_uop,
            repeat_count=repeat,
        )

    if SEGMENTED:
        return [mk("seed"), mk("steady"), mk("step")]
    return [mk("seed"), mk("steady")]


def _register_ops():
    import concourse.dve_ops as dve_ops
    from concourse.dve_spec import (
        Spec, Src0, Src1, C0, C1, C2, scan, AluOp, eq, select,
    )
    from concourse.dve_uop import DveOpSpec

    def reg(name, spec, uops=None, subdim=False):
        """Register op; if `uops` given, inject the hand-built program via
        the compile cache (bypassing lower())."""
        for op in dve_ops.OPS:
            if op.name == name:
                return op
        row = dve_ops._CUSTOM_DVE_ROW_BASE + len(dve_ops.OPS)
        assert row < 0x20, "out of custom-DVE opcode rows"
        shas = {}
        for ver in ("v3", "v4"):
            if uops is not None:
                s = DveOpSpec(name=name, opcode=row, uops=uops, rd1_en=True)
            else:
                from concourse.dve_spec import lower, _has_src1
                s = DveOpSpec(name=name, opcode=row, uops=lower(spec, ver=ver),
                              rd1_en=_has_src1(spec))
            shas[ver] = s.sha(ver)
            if uops is not None:
                dve_ops._COMPILE_CACHE[(name, ver)] = s
        op = dve_ops.DveOp(name, spec, subdim=subdim, uops_sha=shas)
        dve_ops.OPS.append(op)
        dve_ops.CUSTOM_DVE_SPECS[name] = spec
        dve_ops._SUB_OPCODE_FOR_NAME[name] = row
        return op

    def _ref_pair2(in0, in1, s0, s1, imm2):
        a = in0.astype(np.float64)
        b = in1.astype(np.float64)
        d = ((a == s0) * 1.0 + (a == s1) * imm2
             + (b == s0) * 1.0 + (b == s1) * imm2)
        if SEGMENTED:
            # prefix scan resets at each boundary of the innermost dim
            return np.cumsum(d, axis=-1).astype(np.float32)
        return np.cumsum(d.reshape(d.shape[0], -1), axis=1).astype(np.float32)

    d0 = select(eq(Src0, C1), C2, eq(Src0, C0))
    d1 = select(eq(Src1, C1), C2, eq(Src1, C0))
    pair2 = reg("ENT_PAIR2_SCAN",
                Spec(body=scan(AluOp.ADD, d0 + d1), reference=_ref_pair2),
                uops=_build_pair2_uops(), subdim=SEGMENTED)
    return pair2


def _build_nc(repeat=1):
    from contextlib import ExitStack, nullcontext
    import concourse.bacc as bacc
    import concourse.mybir as mybir
    from concourse.tile import TileContext

    PAIR2 = _register_ops()
    dt = mybir.dt
    Alu = mybir.AluOpType
    Act = mybir.ActivationFunctionType

    nc = bacc.Bacc()
    xe = nc.dram_tensor("xe", [ROWS_PC, PAIRS], dt.float16, kind="ExternalInput")
    xo = nc.dram_tensor("xo", [ROWS_PC, PAIRS], dt.float16, kind="ExternalInput")
    y = nc.dram_tensor("y", [ROWS_PC, 1], dt.float32, kind="ExternalOutput")

    # partition p owns rows [p*RPP, (p+1)*RPP); chunk c covers RC rows
    xev = xe[:].rearrange("(p c r) q -> p c (r q)", p=P, c=NCHUNK)
    xov = xo[:].rearrange("(p c r) q -> p c (r q)", p=P, c=NCHUNK)
    yv = y[:].rearrange("(p c r) o -> p c (r o)", p=P, c=NCHUNK)

    NF = RC * PAIRS            # 2048 pairs per partition per chunk
    NA = RC * NL               # 1280 limb accumulators per partition per chunk
    inv_sp = float(1.0 / S_PRIME)

    with TileContext(nc) as tc:
        with ExitStack() as ctx:
            xpool = ctx.enter_context(tc.tile_pool(name="xp", bufs=3))
            ppool = ctx.enter_context(tc.tile_pool(name="pp", bufs=6))
            apool = ctx.enter_context(tc.tile_pool(name="ap", bufs=3))
            dpool = ctx.enter_context(tc.tile_pool(name="dp", bufs=2))
            prpool = ctx.enter_context(tc.tile_pool(name="pr", bufs=3))
            epool = ctx.enter_context(tc.tile_pool(name="ep", bufs=2))
            singles = ctx.enter_context(tc.tile_pool(name="sg", bufs=1))

            t_eps = singles.tile([P, 1], dt.float32)
            nc.vector.memset(t_eps[:], EPS)
            t_inv = singles.tile([P, 1], dt.float32)
            nc.vector.memset(t_inv[:], inv_sp)
            t_mg = singles.tile([P, 1], dt.float32)
            nc.vector.memset(t_mg[:], float(MAGIC - 0.25))
            t_nmg = singles.tile([P, 1], dt.float32)
            nc.vector.memset(t_nmg[:], float(-MAGIC))
            t_invw = singles.tile([P, 1], dt.float32)
            nc.vector.memset(t_invw[:], float(1.0 / W))
            t_one = singles.tile([P, 1], dt.float32)
            nc.vector.memset(t_one[:], 1.0)
            t_mw = singles.tile([P, 1], dt.float32)
            nc.vector.memset(t_mw[:], float(-W))
            t_zero = singles.tile([P, 1], dt.float32)
            nc.vector.memset(t_zero[:], 0.0)

            # Software pipeline, staged so the DVE map stream never waits on
            # the decode chain:
            #   stage_maps(c):  DMA in, 20 DVE map scans, per-limb boundary
            #                   extraction (ACT row-0 copy + gpsimd diff) —
            #                   these release the pref buffers quickly.
            #   stage_mid(c):   emitted after stage_maps(c+1): ACT rint/Ln
            #                   chain + gpsimd c0/products. Keeps chunk c+1's
            #                   diffs AHEAD of chunk c's big gpsimd ops in
            #                   the gpsimd FIFO.
            #   stage_back(c):  emitted after stage_maps(c+2): DVE reduce +
            #                   scale, DMA out. Two chunks of slack so the
            #                   DVE never stalls on decode dependencies.

            def stage_maps(c):
                xet = xpool.tile([P, NF], dt.float16, tag="xe")
                xot = xpool.tile([P, NF], dt.float16, tag="xo")
                nc.sync.dma_start(out=xet[:], in_=xev[:, c, :])
                nc.sync.dma_start(out=xot[:], in_=xov[:, c, :])
                Ab = apool.tile([P, RC, NL], dt.float32, tag="A")
                for g in range(NL):
                    pref = ppool.tile([P, RC, PAIRS], dt.float32, tag="pref")
                    nc.vector._custom_dve(
                        PAIR2,
                        out=pref[:].rearrange("p r q -> p (r q)"),
                        in0=xet[:], in1=xot[:],
                        s0=float(2 * g), s1=float(2 * g + 1), imm2=W)
                    if MAPONLY:
                        continue
                    # row sums from prefix boundary differences
                    nc.scalar.copy(Ab[:, 0:1, g], pref[:, 0:1, PAIRS - 1])
                    nc.gpsimd.tensor_tensor(
                        out=Ab[:, 1:, g],
                        in0=pref[:, 1:, PAIRS - 1],
                        in1=pref[:, :-1, PAIRS - 1],
                        op=Alu.subtract)
                if MAPONLY:
                    eout = epool.tile([P, RC], dt.float32, tag="eout")
                    nc.scalar.copy(eout[:], pref[:, :, PAIRS - 1])
                    nc.sync.dma_start(out=yv[:, c, :], in_=eout[:])
                return Ab

            def stage_mid(c, Ab):
                Af = Ab[:].rearrange("p r g -> p (r g)")       # [P, NA]
                # c1 = rint(A/128 - 0.25) via exact magic rounding on ACT
                t1 = dpool.tile([P, NA], dt.float32, tag="t1")
                c1 = dpool.tile([P, NA], dt.float32, tag="c1")
                nc.scalar.activation(t1[:], Af, Act.Relu,
                                     bias=t_mg[:], scale=t_invw[:])
                nc.scalar.activation(c1[:], t1[:], Act.Relu, bias=t_nmg[:],
                                     scale=t_one[:])
                # c0 = A - 128*c1 (ACT scale + gpsimd add; keep DVE free)
                c1m = dpool.tile([P, NA], dt.float32, tag="c1m")
                nc.scalar.activation(c1m[:], c1[:], Act.Copy, bias=0.0,
                                     scale=float(-W))
                c0 = dpool.tile([P, NA], dt.float32, tag="c0")
                nc.gpsimd.tensor_tensor(out=c0[:], in0=Af, in1=c1m[:],
                                        op=Alu.add)
                # u_i = log(c_i/S' + eps)
                u0 = dpool.tile([P, NA], dt.float32, tag="t1")
                u1 = dpool.tile([P, NA], dt.float32, tag="c1m")
                nc.scalar.activation(u0[:], c0[:], Act.Ln,
                                     bias=t_eps[:], scale=t_inv[:])
                nc.scalar.activation(u1[:], c1[:], Act.Ln,
                                     bias=t_eps[:], scale=t_inv[:])
                # products; both planes in one tile for a single XY reduce
                pr = prpool.tile([P, RC, 2, NL], dt.float32, tag="pr")
                nc.gpsimd.tensor_tensor(
                    out=pr[:, :, 0, :],
                    in0=c0[:].rearrange("p (r g) -> p r g", g=NL),
                    in1=u0[:].rearrange("p (r g) -> p r g", g=NL),
                    op=Alu.mult)
                nc.gpsimd.tensor_tensor(
                    out=pr[:, :, 1, :],
                    in0=c1[:].rearrange("p (r g) -> p r g", g=NL),
                    in1=u1[:].rearrange("p (r g) -> p r g", g=NL),
                    op=Alu.mult)
                return pr

            def stage_back(c, pr):
                eout = epool.tile([P, RC], dt.float32, tag="eout")
                nc.vector.tensor_reduce(
                    out=eout[:], in_=pr[:], axis=mybir.AxisListType.XY,
                    op=Alu.add)
                nc.vector.tensor_scalar_mul(eout[:], eout[:],
                                            float(-1.0 / S_PRIME))
                nc.sync.dma_start(out=yv[:, c, :], in_=eout[:])

            repctx = tc.For_i(0, repeat, 1) if repeat > 1 else nullcontext()
            with repctx:
                mid_q, back_q = [], []
                for c in range(NCHUNK):
                    Ab = stage_maps(c)
                    if MAPONLY:
                        continue
                    mid_q.append((c, Ab))
                    if len(mid_q) > 1:
                        cc, Abc = mid_q.pop(0)
                        back_q.append((cc, stage_mid(cc, Abc)))
                    if len(back_q) > 1:
                        cb, prb = back_q.pop(0)
                        stage_back(cb, prb)
                for cc, Abc in mid_q:
                    back_q.append((cc, stage_mid(cc, Abc)))
                for cb, prb in back_q:
                    stage_back(cb, prb)

    nc.finalize()
    return nc


def _build_runner(repeat=1):
    """Cached jitted 8-core runner (modeled on bass2jax.run_bass_via_pjrt,
    but reusing one jitted executable across calls)."""
    import jax
    from jax.sharding import Mesh, PartitionSpec
    from jax.experimental.shard_map import shard_map
    import concourse.bass2jax as b2j

    nc = _build_nc(repeat=repeat)
    b2j.install_neuronx_cc_hook()

    import concourse.mybir as mybir
    partition_name = (nc.partition_id_tensor.name
                      if nc.partition_id_tensor else None)
    in_names, out_names, out_avals, zero_outs = [], [], [], []
    for alloc in nc.m.functions[0].allocations:
        if not isinstance(alloc, mybir.MemoryLocationSet):
            continue
        name = alloc.memorylocations[0].name
        if alloc.kind == "ExternalInput":
            if name != partition_name:
                in_names.append(name)
        elif alloc.kind == "ExternalOutput":
            shape = tuple(alloc.tensor_shape)
            dtype = mybir.dt.np(alloc.dtype)
            out_names.append(name)
            out_avals.append(jax.core.ShapedArray(shape, dtype))
            zero_outs.append(np.zeros(shape, dtype))
    n_params = len(in_names)
    all_in_names = in_names + out_names
    if partition_name is not None:
        all_in_names = all_in_names + [partition_name]

    def _body(*args):
        operands = list(args)
        if partition_name is not None:
            operands.append(b2j.partition_id_tensor())
        outs = b2j._bass_exec_p.bind(
            *operands,
            out_avals=tuple(out_avals),
            in_names=tuple(all_in_names),
            out_names=tuple(out_names),
            lowering_input_output_aliases=(),
            sim_require_finite=True,
            sim_require_nnan=True,
            nc=nc,
        )
        return tuple(outs)

    devices = jax.devices()[:NCORES]
    mesh = Mesh(np.asarray(devices), ("core",))
    n_outs = len(out_avals)
    sharded = jax.jit(
        shard_map(_body, mesh=mesh,
                  in_specs=(PartitionSpec("core"),) * (n_params + n_outs),
                  out_specs=(PartitionSpec("core"),) * n_outs,
                  check_rep=False),
        donate_argnums=tuple(range(n_params, n_params + n_outs)),
        keep_unused=True,
    )

    def run(*ins) -> np.ndarray:
        zeros = [np.zeros((NCORES * z.shape[0], *z.shape[1:]), z.dtype)
                 for z in zero_outs]
        out = sharded(*ins, *zeros)
        return np.asarray(out[0])

    run.sharded = sharded
    run.zero_outs = zero_outs
    run.mesh = mesh
    run.in_names = in_names
    return run


def prep_inputs(x: np.ndarray):
    """Full input -> device-input tuple (even fp16, odd fp16)."""
    x = np.asarray(x)
    assert x.shape == (B, L), x.shape
    xe = np.ascontiguousarray(x[:, 0::2]).astype(np.float16)
    xo = np.ascontiguousarray(x[:, 1::2]).astype(np.float16)
    return xe, xo


def kernel(x: np.ndarray) -> np.ndarray:
    global _RUNNER
    ins = prep_inputs(x)
    if _RUNNER is None:
        _RUNNER = _build_runner()
    try:
        out = _RUNNER(*ins)
    except Exception:
        # transient device hiccups (NRT exec-unit resets) have been observed
        # once on this fabric; one retry after a short pause recovers.
        import time
        time.sleep(20.0)
        out = _RUNNER(*ins)
    return out.reshape(B, 1).astype(np.float32)


if __name__ == "__main__":
    rng = np.random.default_rng(0)
    xa = rng.integers(0, VOCAB, size=(B, L)).astype(np.int32)
    out = kernel(x=xa)
    cnt = np.zeros((B, VOCAB), np.float64)
    for v in range(VOCAB):
        cnt[:, v] = (xa == v).sum(1)
    p = cnt / S_PRIME
    ref = -(p * np.log(p + EPS)).sum(1, keepdims=True)
    err = np.abs(out - ref).max()
    rel = err / np.abs(ref).max()
    print("selfcheck max abs err:", err, "rel:", rel)
